# revision 37
# baseline (speedup 1.0000x reference)
"""HardMaxAttention Trainium2 Bass kernel.

Reference computation (per batch b):
    Q = x @ W_Q.T            (T, 2)
    K = x @ W_K.T            (T, 2)
    scores = Q @ K.T         (T, T), causal-masked (strict upper tri = -inf)
    idx = argmax(scores, -1) (T,)
    out = x[idx] @ W_V.T     (T, D)   [== take_along_axis(V, idx)]

Sharding: 8 cores = 4 batches x 2 t-parity shards. Core c handles batch
b=c//2 and t-tiles of parity h=c%2. Each core receives x[b] with rows
PERMUTED so its own 16 t-tiles occupy positions 0..2047 and the other
parity's tiles occupy 2048..4095 (see baseline notes below).

Numerics strategy (argmax must match fp32 reference; emulated host-side):
  * QK projection runs as TWO bf16 matmul passes (x split hi/lo into two
    bf16 tensors on host; same DMA bytes as one fp32 tensor) against a
    THREE-term bf16 split of W ([Wh|Wl|W2], M=12 stationary).  The three
    partial results (PSUM partitions 0-3/4-7/8-11) are folded with a
    partition-shift DMA + scalar adds.  Capture error ~2^-18 on x,
    ~2^-26 on W: emulation shows 0 argmax flips vs the fp32 reference.
  * Scores run on float32r (1 cyc/row vs fp32's 4) with an exact hi/lo
    split of q and k absorbed into the contraction: K=8 rows
    qt=[qh0,ql0,qh0,ql0,qh1,ql1,qh1,ql1], kt=[kh0,kh0,kl0,kl0,...].
    hi parts are bf16-rounded (exact under fp32r's fp22 truncation);
    lo parts are ~2^-9 so their fp22 truncation error is ~2^-21.
  * V projection and output are bf16 (affects magnitude only, not the
    gather index); host upcasts the bf16 output to fp32.
Engine placement: PE does proj/scores/V matmuls; Scalar does PSUM->SBUF
score copies (+ causal mask adds fused via scalar.add) and output
copies; DVE does the argmax scans (MAX8/FIND_INDEX8) and small index
math; GpSimd does the gathers; the per-tile xg transposes run on the
XBAR DMA transpose path instead of the PE.
"""

import os
import numpy as np

DEBUG_LEVEL = int(os.environ.get("KERNEL_DEBUG_LEVEL", "99"))
USE_XBAR_TRANSPOSE = os.environ.get("KERNEL_XBAR", "0") == "1"

B, T, D, H = 4, 4096, 1024, 2
P = 128
NT = T // P            # 32 t-tiles per batch
MYT = NT // 2          # 16 t-tiles per core
KD = D // P            # 8 contraction blocks
NG = T // 512          # 8 QK groups
N_CORES = 8
NEG = -1.0e30

_prog_cache = {}


def _build_program():
    """Build the single SPMD Bass program (same for every core)."""
    import concourse.bacc as bacc
    import concourse.mybir as mybir
    import concourse.tile as tile
    import concourse.bass as bass
    from concourse.masks import make_identity

    f32 = mybir.dt.float32
    f32r = mybir.dt.float32r
    bf16 = mybir.dt.bfloat16
    u32 = mybir.dt.uint32

    nc = bacc.Bacc(None, target_bir_lowering=False)

    # x hi/lo bf16 pair in transposed group layout:
    # xq*[g, p, k*512+c] = x_perm[g*512+c, k*128+p]
    xqh = nc.dram_tensor("xqh", [NG, P, KD * 512], bf16, kind="ExternalInput")
    xql = nc.dram_tensor("xql", [NG, P, KD * 512], bf16, kind="ExternalInput")
    # gather + V-projection source (row-major permuted x, bf16)
    xv = nc.dram_tensor("xv", [T, D], bf16, kind="ExternalInput")
    # [D, 12] bf16: cols 0-3 Wh(q0,q1,k0,k1), 4-7 Wl, 8-11 W2
    wtri = nc.dram_tensor("wtri", [D, 12], bf16, kind="ExternalInput")
    # [12, 4] selector: wsel[j*4+m, m] = 1 -- sums the three W-split
    # partial groups of p12 on the PE (exact in fp32)
    wsel = nc.dram_tensor("wsel", [12, 4], f32, kind="ExternalInput")
    w_vT = nc.dram_tensor("w_vT", [D, D], bf16, kind="ExternalInput")
    dmask = nc.dram_tensor("dmask", [P, P], f32, kind="ExternalInput")
    tmask = nc.dram_tensor("tmask", [P, P], f32, kind="ExternalInput")
    out = nc.dram_tensor("out", [MYT, P, D], bf16, kind="ExternalOutput")

    with tile.TileContext(nc) as tc:
        with (
            tc.tile_pool(name="const", bufs=1) as cpool,
            tc.tile_pool(name="xin", bufs=2) as xpool,
            tc.tile_pool(name="qkw", bufs=1) as qkpool,
            tc.tile_pool(name="sc", bufs=2) as scpool,
            tc.tile_pool(name="small", bufs=4) as spool,
            tc.tile_pool(name="xg", bufs=3) as xgpool,
            tc.tile_pool(name="xt", bufs=2) as xtpool,
            tc.tile_pool(name="ob", bufs=3) as opool,
            tc.tile_pool(name="mm_ps", bufs=2, space="PSUM") as mmpsum,
            tc.tile_pool(name="tp_ps", bufs=2, space="PSUM") as tpsum,
            tc.tile_pool(name="vo_ps", bufs=2, space="PSUM") as vopsum,
        ):
            # ---- constants ----
            wcat_sb = cpool.tile([P, 12 * KD], bf16)
            for k in range(KD):
                nc.sync.dma_start(
                    wcat_sb[:, k * 12:(k + 1) * 12], wtri[k * P:(k + 1) * P, :]
                )
            dmask_sb = cpool.tile([P, P], f32)
            nc.sync.dma_start(dmask_sb[:], dmask[:])
            tmask_sb = cpool.tile([P, P], f32)
            nc.sync.dma_start(tmask_sb[:], tmask[:])
            wsel_sb = cpool.tile([12, 4], f32)
            nc.sync.dma_start(wsel_sb[:], wsel[:])
            ident = cpool.tile([P, P], bf16)
            if not USE_XBAR_TRANSPOSE:
                make_identity(nc, ident[:])

            qk = qkpool.tile([4, T], f32, tag="qk")
            hib = qkpool.tile([4, T], bf16, tag="hib")
            hi = qkpool.tile([4, T], f32r, tag="hi")
            lo = qkpool.tile([4, T], f32r, tag="lo")
            # qt rows: [qh0,ql0,qh0,ql0,qh1,ql1,qh1,ql1] over own cols.
            # kt rows: [kh0,kl0,kl0,kh0,kh1,kl1,kl1,kh1] over all cols --
            # the shifted interleave makes every contraction pair
            # qt[r]*kt[r] one of the 4 hi/lo cross products per head, and
            # both tensors assemble from simple stride/broadcast DMAs.
            TQ = T // 2
            qt = qkpool.tile([8, TQ], f32r, tag="qt")
            kt = qkpool.tile([8, T], f32r, tag="kt")
            wv_sb = cpool.tile([P, KD * D], bf16)

            def seg2(t, rows, j):
                """cols [j*512,(j+1)*512) u [2048+j*512, ...): [rows, 2, 512]"""
                return t[rows, :].rearrange(
                    "p (s c) -> p s c", c=512
                )[:, j:j + 5:4, :]

            # ---- phase 1: pairs (own group j, other group 4+j). After a
            # pair, its qk/qt/kt columns are final, so score tiles that only
            # need the first pairs can start while later pairs stream in. ----
            for j in range(NG // 2):
                for g in (j, NG // 2 + j):
                    xh_sb = xpool.tile([P, KD * 512], bf16, tag="xh")
                    xl_sb = xpool.tile([P, KD * 512], bf16, tag="xl")
                    nc.sync.dma_start(xh_sb[:], xqh[g, :, :])
                    nc.scalar.dma_start(xl_sb[:], xql[g, :, :])
                    p12 = mmpsum.tile([12, 512], f32, space="PSUM", tag="mmps")
                    for k in range(KD):
                        for rhs_sb in (xh_sb, xl_sb):
                            nc.tensor.matmul(
                                p12[:],
                                lhsT=wcat_sb[:, k * 12:(k + 1) * 12],
                                rhs=rhs_sb[:, k * 512:(k + 1) * 512],
                                start=(k == 0 and rhs_sb is xh_sb),
                                stop=(k == KD - 1 and rhs_sb is xl_sb),
                            )
                    qkg = xpool.tile([12, 512], f32, tag="qkg")
                    nc.vector.tensor_copy(qkg[:], p12[:])
                    q4 = mmpsum.tile([4, 512], f32, space="PSUM", tag="mmps")
                    nc.tensor.matmul(q4[:], lhsT=wsel_sb[:], rhs=qkg[:],
                                     start=True, stop=True)
                    nc.vector.tensor_copy(qk[:, g * 512:(g + 1) * 512], q4[:])

                # hi/lo split on this pair's two column segments
                nc.scalar.copy(seg2(hib, slice(0, 4), j),
                               seg2(qk, slice(0, 4), j))
                nc.scalar.copy(seg2(hi, slice(0, 4), j),
                               seg2(hib, slice(0, 4), j))
                nc.vector.tensor_tensor(
                    out=seg2(lo, slice(0, 4), j),
                    in0=seg2(qk, slice(0, 4), j),
                    in1=seg2(hi, slice(0, 4), j),
                    op=mybir.AluOpType.subtract,
                )

                # qt assembly (own segment only): even rows <- hi, odd <- lo
                c0, c1 = j * 512, (j + 1) * 512
                nc.sync.dma_start(
                    qt[:, c0:c1].rearrange("(a b) f -> a b f", b=2)[:, 0:1, :],
                    hi[0:2, c0:c1].unsqueeze(1).broadcast_to([2, 2, 512]),
                )
                nc.sync.dma_start(
                    qt[:, c0:c1].rearrange("(a b) f -> a b f", b=2)[:, 1:2, :],
                    lo[0:2, c0:c1].unsqueeze(1).broadcast_to([2, 2, 512]),
                )
                # kt assembly, both segments: per head h rows
                # {4h, 4h+3} <- k_hi(h), {4h+1, 4h+2} <- k_lo(h)
                for h in range(2):
                    for dst_r, src, src_r in (
                        (4 * h + 0, hi, 2 + h),
                        (4 * h + 3, hi, 2 + h),
                        (4 * h + 1, lo, 2 + h),
                        (4 * h + 2, lo, 2 + h),
                    ):
                        nc.scalar.dma_start(
                            seg2(kt, slice(dst_r, dst_r + 1), j),
                            seg2(src, slice(src_r, src_r + 1), j),
                        )

                if j == 0:
                    # W_V^T load queued after the first pair's input DMAs
                    for k in range(KD):
                        nc.sync.dma_start(
                            wv_sb[:, k * D:(k + 1) * D],
                            w_vT[k * P:(k + 1) * P, :],
                        )

            if DEBUG_LEVEL < 3:
                for i in range(MYT):
                    ob = opool.tile([P, D], bf16)
                    nc.vector.memset(ob[:], 0.0)
                    if DEBUG_LEVEL >= 2:
                        nc.vector.tensor_copy(
                            ob[0:8, 0:P], kt[:, i * P:(i + 1) * P]
                        )
                    else:
                        nc.vector.tensor_copy(
                            ob[0:4, 0:P], qk[:, i * P:(i + 1) * P]
                        )
                    nc.sync.dma_start(out[i, :, :], ob[:])

            # ---- phase 2+3 per own t-tile ----
            for i in range(MYT if DEBUG_LEVEL >= 3 else 0):
                E = (i + 1) * P       # width of each of the two key ranges
                W = 2 * E
                sc = scpool.tile([P, 2 * MYT * P], f32)  # max width 4096
                qtile = qt[:, i * P:(i + 1) * P]

                # range A: own-parity keys [0, E), diag block last P cols
                # range B: other-parity keys [2048, 2048+E) -> cols [E, 2E)
                for (base_src, base_dst, mk) in (
                    (0, 0, dmask_sb),
                    (T // 2, E, tmask_sb),
                ):
                    for c0 in range(0, E, 512):
                        c1 = min(E, c0 + 512)
                        nn = c1 - c0
                        ps = mmpsum.tile([P, 512], f32, space="PSUM",
                                         tag="mmps")
                        nc.tensor.matmul(
                            ps[:, :nn],
                            lhsT=qtile,
                            rhs=kt[:, base_src + c0:base_src + c1],
                            start=True,
                            stop=True,
                        )
                        if c1 == E:
                            # chunk contains the masked block (last P cols)
                            if nn > P:
                                nc.scalar.copy(
                                    sc[:, base_dst + c0:base_dst + c1 - P],
                                    ps[:, :nn - P],
                                )
                            nc.vector.tensor_tensor(
                                out=sc[:, base_dst + E - P:base_dst + E],
                                in0=ps[:, nn - P:nn],
                                in1=mk[:],
                                op=mybir.AluOpType.add,
                            )
                        else:
                            nc.scalar.copy(
                                sc[:, base_dst + c0:base_dst + c1], ps[:, :nn]
                            )

                if DEBUG_LEVEL < 4:
                    ob = opool.tile([P, D], bf16)
                    nc.vector.memset(ob[:], 0.0)
                    nc.vector.tensor_copy(ob[:, 0:128], sc[:, 0:128])
                    nc.sync.dma_start(out[i, :, :], ob[:])
                    continue

                mx8 = spool.tile([P, 8], f32, tag="mx8")
                ix8 = spool.tile([P, 8], u32, tag="ix8")
                nc.vector.max(out=mx8[:], in_=sc[:, :W])
                nc.vector.max_index(out=ix8[:], in_max=mx8[:], in_values=sc[:, :W])

                # positions >= E belong to range B: add (2048 - E)
                idxf = spool.tile([P, 1], f32, tag="idxf")
                gef = spool.tile([P, 1], f32, tag="gef")
                idxu = spool.tile([P, 1], u32, tag="idxu")
                nc.vector.tensor_copy(idxf[:], ix8[:, 0:1])
                nc.vector.tensor_scalar(
                    gef[:], idxf[:], float(E), scalar2=float(T // 2 - E),
                    op0=mybir.AluOpType.is_ge, op1=mybir.AluOpType.mult,
                )
                nc.vector.tensor_tensor(
                    out=idxf[:], in0=idxf[:], in1=gef[:],
                    op=mybir.AluOpType.add,
                )
                nc.vector.tensor_copy(idxu[:], idxf[:])

                if DEBUG_LEVEL < 5:
                    ob = opool.tile([P, D], bf16)
                    nc.vector.memset(ob[:], 0.0)
                    # exact idx dump: low byte + high byte (each <=255, exact
                    # in bf16), plus raw max value and raw ix8[0]
                    blo = spool.tile([P, 1], u32, tag="blo")
                    bhi = spool.tile([P, 1], u32, tag="bhi")
                    nc.vector.tensor_scalar(
                        blo[:], idxu[:], 255, scalar2=None,
                        op0=mybir.AluOpType.bitwise_and,
                    )
                    nc.vector.tensor_scalar(
                        bhi[:], idxu[:], 8, scalar2=None,
                        op0=mybir.AluOpType.logical_shift_right,
                    )
                    nc.vector.tensor_copy(ob[:, 0:1], blo[:])
                    nc.vector.tensor_copy(ob[:, 1:2], bhi[:])
                    nc.vector.tensor_copy(ob[:, 2:3], mx8[:, 0:1])
                    nc.sync.dma_start(out[i, :, :], ob[:])
                    continue

                # gather the argmax rows of (permuted) x
                xg = xgpool.tile([P, D], bf16)
                nc.gpsimd.indirect_dma_start(
                    out=xg[:],
                    out_offset=None,
                    in_=xv[:],
                    in_offset=bass.IndirectOffsetOnAxis(ap=idxu[:, 0:1], axis=0),
                )

                if DEBUG_LEVEL < 6:
                    ob = opool.tile([P, D], bf16)
                    nc.vector.tensor_copy(ob[:], xg[:])
                    nc.sync.dma_start(out[i, :, :], ob[:])
                    continue

                # transpose gathered rows
                xgT = xtpool.tile([P, D], bf16, tag="xgt")
                if USE_XBAR_TRANSPOSE:
                    for k in range(KD):
                        nc.sync.dma_start(
                            xgT[:, k * P:(k + 1) * P],
                            xg[:, k * P:(k + 1) * P],
                            transpose=True,
                        )
                else:
                    for k in range(KD):
                        tp = tpsum.tile([P, P], bf16, space="PSUM", tag="tp")
                        nc.tensor.transpose(
                            tp[:], xg[:, k * P:(k + 1) * P], ident[:]
                        )
                        nc.vector.tensor_copy(xgT[:, k * P:(k + 1) * P], tp[:])

                if DEBUG_LEVEL < 7:
                    ob = opool.tile([P, D], bf16)
                    nc.vector.tensor_copy(ob[:], xgT[:])
                    nc.sync.dma_start(out[i, :, :], ob[:])
                    continue

                # out tile = xg @ W_V.T  ->  (xgT).T @ w_vT
                vo = vopsum.tile([P, D], f32, space="PSUM")
                for k in range(KD):
                    for n in range(2):
                        nc.tensor.matmul(
                            vo[:, n * 512:(n + 1) * 512],
                            lhsT=xgT[:, k * P:(k + 1) * P],
                            rhs=wv_sb[:, k * D + n * 512:k * D + n * 512 + 512],
                            start=(k == 0),
                            stop=(k == KD - 1),
                        )
                ob = opool.tile([P, D], bf16)
                nc.scalar.copy(ob[:], vo[:])
                nc.sync.dma_start(out[i, :, :], ob[:])

    nc.compile()
    return nc


def get_program():
    if "nc" not in _prog_cache:
        _prog_cache["nc"] = _build_program()
    return _prog_cache["nc"]


def make_core_inputs(x_full, W_Q, W_K, W_V):
    """Host-side shard: per-core input dicts (and the tile maps)."""
    import ml_dtypes
    bf = ml_dtypes.bfloat16

    x_full = np.ascontiguousarray(x_full, dtype=np.float32)
    WQK = np.concatenate([np.asarray(W_Q, np.float32),
                          np.asarray(W_K, np.float32)], axis=0)  # (4, D)
    Wh = WQK.astype(bf)
    Wl = (WQK - Wh.astype(np.float32)).astype(bf)
    W2 = (WQK - Wh.astype(np.float32) - Wl.astype(np.float32)).astype(bf)
    wtri = np.ascontiguousarray(
        np.concatenate(
            [Wh.T.astype(bf), Wl.T.astype(bf), W2.T.astype(bf)], axis=1
        )
    )  # (D, 12) bf16
    w_vT = np.ascontiguousarray(np.asarray(W_V, np.float32).T.astype(bf))

    r = np.arange(P)
    dmask = np.where(r[None, :] <= r[:, None], 0.0, NEG).astype(np.float32)
    wsel = np.zeros((12, 4), dtype=np.float32)
    for j in range(3):
        for m in range(4):
            wsel[j * 4 + m, m] = 1.0

    def to_groups(a):
        # (T, D) -> [NG, P, KD*512] with a[g, p, k*512+c] = src[g*512+c, k*128+p]
        return np.ascontiguousarray(
            a.reshape(NG, 512, KD, P).transpose(0, 3, 2, 1)
            .reshape(NG, P, KD * 512)
        )

    in_maps = []
    tiles_per_core = []
    for c in range(N_CORES):
        b, h = divmod(c, 2)
        mine = [2 * i + h for i in range(MYT)]
        other = [2 * i + (1 - h) for i in range(MYT)]
        rows = np.concatenate(
            [np.arange(t * P, (t + 1) * P) for t in mine + other]
        )
        xb_perm = np.ascontiguousarray(x_full[b][rows])
        xh = xb_perm.astype(bf)
        xl = (xb_perm - xh.astype(np.float32)).astype(bf)
        # other-parity tile at position 2048+i*128 is true block 2i+(1-h):
        # h=0 -> block 2i+1 > diag 2i   -> fully masked
        # h=1 -> block 2i   < diag 2i+1 -> fully valid
        tmask = np.full((P, P), NEG if h == 0 else 0.0, dtype=np.float32)
        in_maps.append(
            {
                "xqh": to_groups(xh),
                "xql": to_groups(xl),
                "xv": np.ascontiguousarray(xb_perm.astype(bf)),
                "wtri": wtri,
                "wsel": wsel,
                "w_vT": w_vT,
                "dmask": dmask,
                "tmask": tmask,
            }
        )
        tiles_per_core.append(mine)
    return in_maps, tiles_per_core


def assemble_output(results, tiles_per_core):
    out_full = np.empty((B, T, D), dtype=np.float32)
    for c in range(N_CORES):
        b = c // 2
        oc = results[c]["out"].astype(np.float32)
        for i, th in enumerate(tiles_per_core[c]):
            out_full[b, th * P:(th + 1) * P, :] = oc[i]
    return out_full


def kernel(**inputs):
    from concourse.bass_utils import run_bass_kernel_spmd

    x_full = np.asarray(inputs["x"], dtype=np.float32)
    in_maps, tiles_per_core = make_core_inputs(
        x_full, np.asarray(inputs["W_Q"]), np.asarray(inputs["W_K"]),
        np.asarray(inputs["W_V"])
    )
    nc = get_program()
    res = run_bass_kernel_spmd(nc, in_maps, core_ids=list(range(N_CORES)))
    return assemble_output(res.results, tiles_per_core)


# revision 40
# speedup vs baseline: 1.0132x; 1.0132x over previous
"""HardMaxAttention Trainium2 Bass kernel.

Reference computation (per batch b):
    Q = x @ W_Q.T            (T, 2)
    K = x @ W_K.T            (T, 2)
    scores = Q @ K.T         (T, T), causal-masked (strict upper tri = -inf)
    idx = argmax(scores, -1) (T,)
    out = x[idx] @ W_V.T     (T, D)   [== take_along_axis(V, idx)]

Sharding: 8 cores = 4 batches x 2 t-parity shards. Core c handles batch
b=c//2 and t-tiles of parity h=c%2. Each core receives x[b] with rows
PERMUTED so its own 16 t-tiles occupy positions 0..2047 and the other
parity's tiles occupy 2048..4095 (see baseline notes below).

Numerics strategy (argmax must match fp32 reference; emulated host-side):
  * QK projection runs as TWO bf16 matmul passes (x split hi/lo into two
    bf16 tensors on host; same DMA bytes as one fp32 tensor) against a
    THREE-term bf16 split of W ([Wh|Wl|W2], M=12 stationary).  The three
    partial results (PSUM partitions 0-3/4-7/8-11) are folded with a
    partition-shift DMA + scalar adds.  Capture error ~2^-18 on x,
    ~2^-26 on W: emulation shows 0 argmax flips vs the fp32 reference.
  * Scores run on float32r (1 cyc/row vs fp32's 4) with an exact hi/lo
    split of q and k absorbed into the contraction: K=8 rows
    qt=[qh0,ql0,qh0,ql0,qh1,ql1,qh1,ql1], kt=[kh0,kh0,kl0,kl0,...].
    hi parts are bf16-rounded (exact under fp32r's fp22 truncation);
    lo parts are ~2^-9 so their fp22 truncation error is ~2^-21.
  * V projection and output are bf16 (affects magnitude only, not the
    gather index); host upcasts the bf16 output to fp32.
Engine placement: PE does proj/scores/V matmuls; Scalar does PSUM->SBUF
score copies (+ causal mask adds fused via scalar.add) and output
copies; DVE does the argmax scans (MAX8/FIND_INDEX8) and small index
math; GpSimd does the gathers; the per-tile xg transposes run on the
XBAR DMA transpose path instead of the PE.
"""

import os
import numpy as np

DEBUG_LEVEL = int(os.environ.get("KERNEL_DEBUG_LEVEL", "99"))
USE_XBAR_TRANSPOSE = os.environ.get("KERNEL_XBAR", "0") == "1"

B, T, D, H = 4, 4096, 1024, 2
P = 128
NT = T // P            # 32 t-tiles per batch
MYT = NT // 2          # 16 t-tiles per core
KD = D // P            # 8 contraction blocks
NG = T // 512          # 8 QK groups
N_CORES = 8
NEG = -1.0e30

_prog_cache = {}


def _build_program():
    """Build the single SPMD Bass program (same for every core)."""
    import concourse.bacc as bacc
    import concourse.mybir as mybir
    import concourse.tile as tile
    import concourse.bass as bass
    from concourse.masks import make_identity

    f32 = mybir.dt.float32
    f32r = mybir.dt.float32r
    bf16 = mybir.dt.bfloat16
    u32 = mybir.dt.uint32

    nc = bacc.Bacc(None, target_bir_lowering=False)

    # x hi/lo bf16 pair in transposed group layout:
    # xq*[g, p, k*512+c] = x_perm[g*512+c, k*128+p]
    xqh = nc.dram_tensor("xqh", [NG, P, KD * 512], bf16, kind="ExternalInput")
    xql = nc.dram_tensor("xql", [NG, P, KD * 512], bf16, kind="ExternalInput")
    # gather + V-projection source (row-major permuted x, bf16)
    xv = nc.dram_tensor("xv", [T, D], bf16, kind="ExternalInput")
    # [D, 12] bf16: cols 0-3 Wh(q0,q1,k0,k1), 4-7 Wl, 8-11 W2
    wtri = nc.dram_tensor("wtri", [D, 12], bf16, kind="ExternalInput")
    # [12, 4] selector: wsel[j*4+m, m] = 1 -- sums the three W-split
    # partial groups of p12 on the PE (exact in fp32)
    wsel = nc.dram_tensor("wsel", [12, 4], f32, kind="ExternalInput")
    w_vT = nc.dram_tensor("w_vT", [D, D], bf16, kind="ExternalInput")
    dmask = nc.dram_tensor("dmask", [P, P], f32, kind="ExternalInput")
    tmask = nc.dram_tensor("tmask", [P, P], f32, kind="ExternalInput")
    out = nc.dram_tensor("out", [MYT, P, D], bf16, kind="ExternalOutput")

    with tile.TileContext(nc) as tc:
        with (
            tc.tile_pool(name="const", bufs=1) as cpool,
            tc.tile_pool(name="xin", bufs=2) as xpool,
            tc.tile_pool(name="qkw", bufs=1) as qkpool,
            tc.tile_pool(name="sc", bufs=2) as scpool,
            tc.tile_pool(name="small", bufs=4) as spool,
            tc.tile_pool(name="xg", bufs=3) as xgpool,
            tc.tile_pool(name="xt", bufs=2) as xtpool,
            tc.tile_pool(name="ob", bufs=3) as opool,
            tc.tile_pool(name="mm_ps", bufs=2, space="PSUM") as mmpsum,
            tc.tile_pool(name="tp_ps", bufs=2, space="PSUM") as tpsum,
            tc.tile_pool(name="vo_ps", bufs=2, space="PSUM") as vopsum,
        ):
            # ---- constants ----
            wcat_sb = cpool.tile([P, 12 * KD], bf16)
            for k in range(KD):
                nc.sync.dma_start(
                    wcat_sb[:, k * 12:(k + 1) * 12], wtri[k * P:(k + 1) * P, :]
                )
            dmask_sb = cpool.tile([P, P], f32)
            nc.sync.dma_start(dmask_sb[:], dmask[:])
            tmask_sb = cpool.tile([P, P], f32)
            nc.sync.dma_start(tmask_sb[:], tmask[:])
            wsel_sb = cpool.tile([12, 4], f32)
            nc.sync.dma_start(wsel_sb[:], wsel[:])
            ident = cpool.tile([P, P], bf16)
            if not USE_XBAR_TRANSPOSE:
                make_identity(nc, ident[:])

            qk = qkpool.tile([4, T], f32, tag="qk")
            hib = qkpool.tile([4, T], bf16, tag="hib")
            hi = qkpool.tile([4, T], f32r, tag="hi")
            lo = qkpool.tile([4, T], f32r, tag="lo")
            # qt rows: [qh0,ql0,qh0,ql0,qh1,ql1,qh1,ql1] over own cols.
            # kt rows: [kh0,kl0,kl0,kh0,kh1,kl1,kl1,kh1] over all cols --
            # the shifted interleave makes every contraction pair
            # qt[r]*kt[r] one of the 4 hi/lo cross products per head, and
            # both tensors assemble from simple stride/broadcast DMAs.
            TQ = T // 2
            qt = qkpool.tile([8, TQ], f32r, tag="qt")
            kt = qkpool.tile([8, T], f32r, tag="kt")
            wv_sb = cpool.tile([P, KD * D], bf16)

            def seg2(t, rows, j):
                """cols [j*512,(j+1)*512) u [2048+j*512, ...): [rows, 2, 512]"""
                return t[rows, :].rearrange(
                    "p (s c) -> p s c", c=512
                )[:, j:j + 5:4, :]

            def emit_tile(i):
                E = (i + 1) * P       # width of each of the two key ranges
                W = 2 * E
                sc = scpool.tile([P, 2 * MYT * P], f32)  # max width 4096
                qtile = qt[:, i * P:(i + 1) * P]

                # range A: own-parity keys [0, E), diag block last P cols
                # range B: other-parity keys [2048, 2048+E) -> cols [E, 2E)
                for (base_src, base_dst, mk) in (
                    (0, 0, dmask_sb),
                    (T // 2, E, tmask_sb),
                ):
                    for c0 in range(0, E, 512):
                        c1 = min(E, c0 + 512)
                        nn = c1 - c0
                        ps = mmpsum.tile([P, 512], f32, space="PSUM",
                                         tag="mmps")
                        nc.tensor.matmul(
                            ps[:, :nn],
                            lhsT=qtile,
                            rhs=kt[:, base_src + c0:base_src + c1],
                            start=True,
                            stop=True,
                        )
                        if c1 == E:
                            # chunk contains the masked block (last P cols)
                            if nn > P:
                                nc.scalar.copy(
                                    sc[:, base_dst + c0:base_dst + c1 - P],
                                    ps[:, :nn - P],
                                )
                            nc.vector.tensor_tensor(
                                out=sc[:, base_dst + E - P:base_dst + E],
                                in0=ps[:, nn - P:nn],
                                in1=mk[:],
                                op=mybir.AluOpType.add,
                            )
                        else:
                            nc.scalar.copy(
                                sc[:, base_dst + c0:base_dst + c1], ps[:, :nn]
                            )

                if DEBUG_LEVEL < 4:
                    ob = opool.tile([P, D], bf16)
                    nc.vector.memset(ob[:], 0.0)
                    nc.vector.tensor_copy(ob[:, 0:128], sc[:, 0:128])
                    nc.sync.dma_start(out[i, :, :], ob[:])
                    return

                mx8 = spool.tile([P, 8], f32, tag="mx8")
                ix8 = spool.tile([P, 8], u32, tag="ix8")
                nc.vector.max(out=mx8[:], in_=sc[:, :W])
                nc.vector.max_index(out=ix8[:], in_max=mx8[:], in_values=sc[:, :W])

                # positions >= E belong to range B: add (2048 - E)
                idxf = spool.tile([P, 1], f32, tag="idxf")
                gef = spool.tile([P, 1], f32, tag="gef")
                idxu = spool.tile([P, 1], u32, tag="idxu")
                nc.vector.tensor_copy(idxf[:], ix8[:, 0:1])
                nc.vector.tensor_scalar(
                    gef[:], idxf[:], float(E), scalar2=float(T // 2 - E),
                    op0=mybir.AluOpType.is_ge, op1=mybir.AluOpType.mult,
                )
                nc.vector.tensor_tensor(
                    out=idxf[:], in0=idxf[:], in1=gef[:],
                    op=mybir.AluOpType.add,
                )
                nc.vector.tensor_copy(idxu[:], idxf[:])

                if DEBUG_LEVEL < 5:
                    ob = opool.tile([P, D], bf16)
                    nc.vector.memset(ob[:], 0.0)
                    # exact idx dump: low byte + high byte (each <=255, exact
                    # in bf16), plus raw max value and raw ix8[0]
                    blo = spool.tile([P, 1], u32, tag="blo")
                    bhi = spool.tile([P, 1], u32, tag="bhi")
                    nc.vector.tensor_scalar(
                        blo[:], idxu[:], 255, scalar2=None,
                        op0=mybir.AluOpType.bitwise_and,
                    )
                    nc.vector.tensor_scalar(
                        bhi[:], idxu[:], 8, scalar2=None,
                        op0=mybir.AluOpType.logical_shift_right,
                    )
                    nc.vector.tensor_copy(ob[:, 0:1], blo[:])
                    nc.vector.tensor_copy(ob[:, 1:2], bhi[:])
                    nc.vector.tensor_copy(ob[:, 2:3], mx8[:, 0:1])
                    nc.sync.dma_start(out[i, :, :], ob[:])
                    return

                # gather the argmax rows of (permuted) x
                xg = xgpool.tile([P, D], bf16)
                nc.gpsimd.indirect_dma_start(
                    out=xg[:],
                    out_offset=None,
                    in_=xv[:],
                    in_offset=bass.IndirectOffsetOnAxis(ap=idxu[:, 0:1], axis=0),
                )

                if DEBUG_LEVEL < 6:
                    ob = opool.tile([P, D], bf16)
                    nc.vector.tensor_copy(ob[:], xg[:])
                    nc.sync.dma_start(out[i, :, :], ob[:])
                    return

                # transpose gathered rows
                xgT = xtpool.tile([P, D], bf16, tag="xgt")
                if USE_XBAR_TRANSPOSE:
                    for k in range(KD):
                        nc.sync.dma_start(
                            xgT[:, k * P:(k + 1) * P],
                            xg[:, k * P:(k + 1) * P],
                            transpose=True,
                        )
                else:
                    for k in range(KD):
                        tp = tpsum.tile([P, P], bf16, space="PSUM", tag="tp")
                        nc.tensor.transpose(
                            tp[:], xg[:, k * P:(k + 1) * P], ident[:]
                        )
                        nc.vector.tensor_copy(xgT[:, k * P:(k + 1) * P], tp[:])

                if DEBUG_LEVEL < 7:
                    ob = opool.tile([P, D], bf16)
                    nc.vector.tensor_copy(ob[:], xgT[:])
                    nc.sync.dma_start(out[i, :, :], ob[:])
                    return

                # out tile = xg @ W_V.T  ->  (xgT).T @ w_vT
                vo = vopsum.tile([P, D], f32, space="PSUM")
                for k in range(KD):
                    for n in range(2):
                        nc.tensor.matmul(
                            vo[:, n * 512:(n + 1) * 512],
                            lhsT=xgT[:, k * P:(k + 1) * P],
                            rhs=wv_sb[:, k * D + n * 512:k * D + n * 512 + 512],
                            start=(k == 0),
                            stop=(k == KD - 1),
                        )
                ob = opool.tile([P, D], bf16)
                nc.scalar.copy(ob[:], vo[:])
                nc.sync.dma_start(out[i, :, :], ob[:])

            # ---- phase 1: pairs (own group j, other group 4+j). After a
            # pair, its qk/qt/kt columns are final, so score tiles that only
            # need the first pairs can start while later pairs stream in. ----
            for j in range(NG // 2):
                for g in (j, NG // 2 + j):
                    xh_sb = xpool.tile([P, KD * 512], bf16, tag="xh")
                    xl_sb = xpool.tile([P, KD * 512], bf16, tag="xl")
                    nc.sync.dma_start(xh_sb[:], xqh[g, :, :])
                    nc.scalar.dma_start(xl_sb[:], xql[g, :, :])
                    p12 = mmpsum.tile([12, 512], f32, space="PSUM", tag="mmps")
                    for k in range(KD):
                        for rhs_sb in (xh_sb, xl_sb):
                            nc.tensor.matmul(
                                p12[:],
                                lhsT=wcat_sb[:, k * 12:(k + 1) * 12],
                                rhs=rhs_sb[:, k * 512:(k + 1) * 512],
                                start=(k == 0 and rhs_sb is xh_sb),
                                stop=(k == KD - 1 and rhs_sb is xl_sb),
                            )
                    qkg = xpool.tile([12, 512], f32, tag="qkg")
                    nc.vector.tensor_copy(qkg[:], p12[:])
                    q4 = mmpsum.tile([4, 512], f32, space="PSUM", tag="mmps")
                    nc.tensor.matmul(q4[:], lhsT=wsel_sb[:], rhs=qkg[:],
                                     start=True, stop=True)
                    nc.vector.tensor_copy(qk[:, g * 512:(g + 1) * 512], q4[:])

                # hi/lo split on this pair's two column segments
                nc.scalar.copy(seg2(hib, slice(0, 4), j),
                               seg2(qk, slice(0, 4), j))
                nc.scalar.copy(seg2(hi, slice(0, 4), j),
                               seg2(hib, slice(0, 4), j))
                nc.vector.tensor_tensor(
                    out=seg2(lo, slice(0, 4), j),
                    in0=seg2(qk, slice(0, 4), j),
                    in1=seg2(hi, slice(0, 4), j),
                    op=mybir.AluOpType.subtract,
                )

                # qt assembly (own segment only): even rows <- hi, odd <- lo
                c0, c1 = j * 512, (j + 1) * 512
                nc.sync.dma_start(
                    qt[:, c0:c1].rearrange("(a b) f -> a b f", b=2)[:, 0:1, :],
                    hi[0:2, c0:c1].unsqueeze(1).broadcast_to([2, 2, 512]),
                )
                nc.sync.dma_start(
                    qt[:, c0:c1].rearrange("(a b) f -> a b f", b=2)[:, 1:2, :],
                    lo[0:2, c0:c1].unsqueeze(1).broadcast_to([2, 2, 512]),
                )
                # kt assembly, both segments: per head h rows
                # {4h, 4h+3} <- k_hi(h), {4h+1, 4h+2} <- k_lo(h)
                for h in range(2):
                    for dst_r, src, src_r in (
                        (4 * h + 0, hi, 2 + h),
                        (4 * h + 3, hi, 2 + h),
                        (4 * h + 1, lo, 2 + h),
                        (4 * h + 2, lo, 2 + h),
                    ):
                        nc.scalar.dma_start(
                            seg2(kt, slice(dst_r, dst_r + 1), j),
                            seg2(src, slice(src_r, src_r + 1), j),
                        )

                if j == 0:
                    # W_V^T load queued after the first pair's input DMAs
                    for k in range(KD):
                        nc.sync.dma_start(
                            wv_sb[:, k * D:(k + 1) * D],
                            w_vT[k * P:(k + 1) * P, :],
                        )

                # tiles 4j..4j+3 only need key pairs 0..j: emit them now so
                # their compute overlaps the remaining pairs' input streams
                for i in range(4 * j, 4 * j + 4):
                    emit_tile(i)

    nc.compile()
    return nc


def get_program():
    if "nc" not in _prog_cache:
        _prog_cache["nc"] = _build_program()
    return _prog_cache["nc"]


def make_core_inputs(x_full, W_Q, W_K, W_V):
    """Host-side shard: per-core input dicts (and the tile maps)."""
    import ml_dtypes
    bf = ml_dtypes.bfloat16

    x_full = np.ascontiguousarray(x_full, dtype=np.float32)
    WQK = np.concatenate([np.asarray(W_Q, np.float32),
                          np.asarray(W_K, np.float32)], axis=0)  # (4, D)
    Wh = WQK.astype(bf)
    Wl = (WQK - Wh.astype(np.float32)).astype(bf)
    W2 = (WQK - Wh.astype(np.float32) - Wl.astype(np.float32)).astype(bf)
    wtri = np.ascontiguousarray(
        np.concatenate(
            [Wh.T.astype(bf), Wl.T.astype(bf), W2.T.astype(bf)], axis=1
        )
    )  # (D, 12) bf16
    w_vT = np.ascontiguousarray(np.asarray(W_V, np.float32).T.astype(bf))

    r = np.arange(P)
    dmask = np.where(r[None, :] <= r[:, None], 0.0, NEG).astype(np.float32)
    wsel = np.zeros((12, 4), dtype=np.float32)
    for j in range(3):
        for m in range(4):
            wsel[j * 4 + m, m] = 1.0

    def to_groups(a):
        # (T, D) -> [NG, P, KD*512] with a[g, p, k*512+c] = src[g*512+c, k*128+p]
        return np.ascontiguousarray(
            a.reshape(NG, 512, KD, P).transpose(0, 3, 2, 1)
            .reshape(NG, P, KD * 512)
        )

    in_maps = []
    tiles_per_core = []
    for c in range(N_CORES):
        b, h = divmod(c, 2)
        mine = [2 * i + h for i in range(MYT)]
        other = [2 * i + (1 - h) for i in range(MYT)]
        rows = np.concatenate(
            [np.arange(t * P, (t + 1) * P) for t in mine + other]
        )
        xb_perm = np.ascontiguousarray(x_full[b][rows])
        xh = xb_perm.astype(bf)
        xl = (xb_perm - xh.astype(np.float32)).astype(bf)
        # other-parity tile at position 2048+i*128 is true block 2i+(1-h):
        # h=0 -> block 2i+1 > diag 2i   -> fully masked
        # h=1 -> block 2i   < diag 2i+1 -> fully valid
        tmask = np.full((P, P), NEG if h == 0 else 0.0, dtype=np.float32)
        in_maps.append(
            {
                "xqh": to_groups(xh),
                "xql": to_groups(xl),
                "xv": np.ascontiguousarray(xb_perm.astype(bf)),
                "wtri": wtri,
                "wsel": wsel,
                "w_vT": w_vT,
                "dmask": dmask,
                "tmask": tmask,
            }
        )
        tiles_per_core.append(mine)
    return in_maps, tiles_per_core


def assemble_output(results, tiles_per_core):
    out_full = np.empty((B, T, D), dtype=np.float32)
    for c in range(N_CORES):
        b = c // 2
        oc = results[c]["out"].astype(np.float32)
        for i, th in enumerate(tiles_per_core[c]):
            out_full[b, th * P:(th + 1) * P, :] = oc[i]
    return out_full


def kernel(**inputs):
    from concourse.bass_utils import run_bass_kernel_spmd

    x_full = np.asarray(inputs["x"], dtype=np.float32)
    in_maps, tiles_per_core = make_core_inputs(
        x_full, np.asarray(inputs["W_Q"]), np.asarray(inputs["W_K"]),
        np.asarray(inputs["W_V"])
    )
    nc = get_program()
    res = run_bass_kernel_spmd(nc, in_maps, core_ids=list(range(N_CORES)))
    return assemble_output(res.results, tiles_per_core)
